# revision 2
# baseline (speedup 1.0000x reference)
"""Trainium2 Bass kernel for DiamondLayer.

Computes out[b, d] = mean(x[b, d:d+16, d+17:d+33]) for d in [0, 2016),
i.e. 16x16 mean-pool windows sliding along a diagonal.

Strategy:
  - Pure data parallel over batch: 32 batches -> 8 cores x 4 batches.
  - Only a narrow diagonal band of each 2048x2048 matrix is touched
    (cols [r+2, r+33] of row r).  Each core DMAs just that band
    (~500KB/batch instead of 16MB/batch) straight out of the full
    input in HBM using a strided access pattern.
  - Band layout: BT[q, tau, j] = x[b, 16q+tau, (16q+tau)+2+j],
    q in [0,126), tau in [0,31), j in [0,32).  Window d = 16q+u needs
    rows 16q+u .. 16q+u+15, all resident in partition q, so the whole
    reduction is per-partition with uniform access patterns:
      out[16q+u] = sum_{i,k} BT[q, u+i, 15-i+k]
    computed as a 2-level pairwise tree (w2, w4) along j followed by
    4 strided window reduces and a final scaled add.
"""

import os
import sys
import numpy as np

for _p in ("/opt/trn_rl_repo",):
    if _p not in sys.path:
        sys.path.insert(0, _p)

B_FULL = 32
N_CORES = 8
B_PER_CORE = B_FULL // N_CORES  # 4
MAT = 2048
DS = 16
ND = MAT - 2 * DS  # 2016
NQ = ND // 16  # 126 partition groups of 16 diamonds
ROW_STRIDE = MAT + 1  # 2049: distance between band starts of adjacent rows
MAT_ELEMS = MAT * MAT

LAST_EXEC_TIME_NS = None

_COMPILED = None


def _build():
    import concourse.bass as bass
    import concourse.bacc as bacc
    import concourse.tile as tile
    from concourse import mybir

    f32 = mybir.dt.float32
    add = mybir.AluOpType.add
    X = mybir.AxisListType.X

    nc = bacc.Bacc("TRN2", target_bir_lowering=False, debug=False)
    x = nc.dram_tensor("x", [B_PER_CORE, MAT, MAT], f32, kind="ExternalInput")
    y = nc.dram_tensor("y", [B_PER_CORE, ND], f32, kind="ExternalOutput")

    def v(t, off, pat):
        return bass.AP(t.tensor, off, pat)

    with tile.TileContext(nc) as tc:
        with (
            tc.tile_pool(name="bt", bufs=2) as p_bt,
            tc.tile_pool(name="w2", bufs=2) as p_w2,
            tc.tile_pool(name="w4", bufs=2) as p_w4,
            tc.tile_pool(name="r", bufs=2) as p_r,
            tc.tile_pool(name="out", bufs=1) as p_out,
        ):
            out_t = p_out.tile([NQ, B_PER_CORE * 16], f32)
            for b in range(B_PER_CORE):
                # band load: BT[q, tau, j] = x[b, 16q+tau, 16q+tau+2+j]
                bt = p_bt.tile([NQ, 31 * 32], f32, tag="bt")
                nc.sync.dma_start(
                    out=v(bt, 0, [[992, NQ], [32, 31], [1, 32]]),
                    in_=bass.AP(
                        x,
                        b * MAT_ELEMS + 2,
                        [[16 * ROW_STRIDE, NQ], [ROW_STRIDE, 31], [1, 32]],
                    ),
                )
                # w2[q, tau, c] = BT[c] + BT[c+1], c in [0, 31)
                w2 = p_w2.tile([NQ, 31 * 32], f32, tag="w2")
                nc.vector.tensor_tensor(
                    out=v(w2, 0, [[992, NQ], [32, 31], [1, 31]]),
                    in0=v(bt, 0, [[992, NQ], [32, 31], [1, 31]]),
                    in1=v(bt, 1, [[992, NQ], [32, 31], [1, 31]]),
                    op=add,
                )
                # w4[q, tau, c] = w2[c] + w2[c+2], c in [0, 29)
                w4 = p_w4.tile([NQ, 31 * 32], f32, tag="w4")
                nc.vector.tensor_tensor(
                    out=v(w4, 0, [[992, NQ], [32, 31], [1, 29]]),
                    in0=v(w2, 0, [[992, NQ], [32, 31], [1, 29]]),
                    in1=v(w2, 2, [[992, NQ], [32, 31], [1, 29]]),
                    op=add,
                )
                # window sums: out[16q+u] = sum_{a in {0,4,8,12}} sum_s w4[q, 32(u+s) + 15-s+a]
                rs = []
                for a in (0, 4, 8, 12):
                    r = p_r.tile([NQ, 16], f32, tag=f"r{a}")
                    nc.vector.reduce_sum(
                        out=r[:, :],
                        in_=v(w4, 15 + a, [[992, NQ], [32, 16], [31, 16]]),
                        axis=X,
                    )
                    rs.append(r)
                t01 = p_r.tile([NQ, 16], f32, tag="t01")
                nc.vector.tensor_tensor(out=t01[:, :], in0=rs[0][:, :], in1=rs[1][:, :], op=add)
                t23 = p_r.tile([NQ, 16], f32, tag="t23")
                nc.vector.tensor_tensor(out=t23[:, :], in0=rs[2][:, :], in1=rs[3][:, :], op=add)
                tsum = p_r.tile([NQ, 16], f32, tag="tsum")
                nc.vector.tensor_tensor(out=tsum[:, :], in0=t01[:, :], in1=t23[:, :], op=add)
                # scale 1/256 into the output staging tile
                nc.scalar.mul(out_t[:, b * 16 : (b + 1) * 16], tsum[:, :], 1.0 / 256.0)

            # y[b, 16q+u] = out_t[q, 16b+u]
            nc.sync.dma_start(
                out=bass.AP(y, 0, [[16, NQ], [ND, B_PER_CORE], [1, 16]]),
                in_=v(out_t, 0, [[64, NQ], [16, B_PER_CORE], [1, 16]]),
            )

    nc.compile()
    return nc


def _get_compiled():
    global _COMPILED
    if _COMPILED is None:
        _COMPILED = _build()
    return _COMPILED


def kernel(x: np.ndarray) -> np.ndarray:
    global LAST_EXEC_TIME_NS
    from concourse.bass_utils import run_bass_kernel_spmd

    x = np.ascontiguousarray(np.asarray(x), dtype=np.float32)
    assert x.shape == (B_FULL, MAT, MAT), x.shape

    nc = _get_compiled()
    in_maps = [
        {"x": x[i * B_PER_CORE : (i + 1) * B_PER_CORE]} for i in range(N_CORES)
    ]
    trace = bool(int(os.environ.get("KERNEL_TRACE", "0")))
    if trace:
        # test-only: keep NTFF artifacts local instead of uploading
        from concourse import bass_utils as _bu

        _bu.upload_artifacts = lambda tmpdir: tmpdir
    res = run_bass_kernel_spmd(
        nc, in_maps, core_ids=list(range(N_CORES)), trace=trace
    )
    LAST_EXEC_TIME_NS = res.exec_time_ns
    out = np.concatenate([res.results[i]["y"] for i in range(N_CORES)], axis=0)
    return out.astype(np.float32)


# revision 3
# speedup vs baseline: 1.0940x; 1.0940x over previous
"""Trainium2 Bass kernel for DiamondLayer.

Computes out[b, d] = mean(x[b, d:d+16, d+17:d+33]) for d in [0, 2016),
i.e. 16x16 mean-pool windows sliding along a diagonal.

Strategy:
  - Pure data parallel over batch: 32 batches -> 8 cores x 4 batches.
  - Only a narrow diagonal band of each 2048x2048 matrix is touched
    (cols [r+2, r+33] of row r).  Each core DMAs just that band
    (~500KB/batch instead of 16MB/batch) straight out of the full
    input in HBM using a strided access pattern.
  - Band layout: BT[q, tau, j] = x[b, 16q+tau, (16q+tau)+2+j],
    q in [0,126), tau in [0,31), j in [0,32).  Window d = 16q+u needs
    rows 16q+u .. 16q+u+15, all resident in partition q, so the whole
    reduction is per-partition with uniform access patterns:
      out[16q+u] = sum_{i,k} BT[q, u+i, 15-i+k]
    computed as a 2-level pairwise tree (w2, w4) along j followed by
    4 strided window reduces and a final scaled add.
"""

import os
import sys
import numpy as np

for _p in ("/opt/trn_rl_repo",):
    if _p not in sys.path:
        sys.path.insert(0, _p)

B_FULL = 32
N_CORES = 8
B_PER_CORE = B_FULL // N_CORES  # 4
MAT = 2048
DS = 16
ND = MAT - 2 * DS  # 2016
NQ = ND // 16  # 126 partition groups of 16 diamonds
ROW_STRIDE = MAT + 1  # 2049: distance between band starts of adjacent rows
MAT_ELEMS = MAT * MAT

LAST_EXEC_TIME_NS = None

_COMPILED = None


def _build():
    import concourse.bass as bass
    import concourse.bacc as bacc
    import concourse.tile as tile
    from concourse import mybir

    f32 = mybir.dt.float32
    add = mybir.AluOpType.add
    X = mybir.AxisListType.X

    nc = bacc.Bacc("TRN2", target_bir_lowering=False, debug=False)
    x = nc.dram_tensor("x", [B_PER_CORE, MAT, MAT], f32, kind="ExternalInput")
    y = nc.dram_tensor("y", [B_PER_CORE, ND], f32, kind="ExternalOutput")

    def v(t, off, pat):
        return bass.AP(t.tensor, off, pat)

    sub = mybir.AluOpType.subtract
    bypass = mybir.AluOpType.bypass

    with tile.TileContext(nc) as tc:
        with (
            tc.tile_pool(name="bt", bufs=2) as p_bt,
            tc.tile_pool(name="pp", bufs=2) as p_pp,
            tc.tile_pool(name="s16", bufs=2) as p_s16,
            tc.tile_pool(name="r", bufs=2) as p_r,
        ):
            for b in range(B_PER_CORE):
                # band load: BT[q, 1 + 32*tau + j] = x[b, 16q+tau, 16q+tau+2+j]
                # split at tau=16 so the scan of the first half can start
                # while the second half is still in flight.
                bt = p_bt.tile([NQ, 1024], f32, tag="bt")
                nc.sync.dma_start(
                    out=v(bt, 1, [[1024, NQ], [32, 16], [1, 32]]),
                    in_=bass.AP(
                        x,
                        b * MAT_ELEMS + 2,
                        [[16 * ROW_STRIDE, NQ], [ROW_STRIDE, 16], [1, 32]],
                    ),
                )
                nc.sync.dma_start(
                    out=v(bt, 1 + 512, [[1024, NQ], [32, 15], [1, 32]]),
                    in_=bass.AP(
                        x,
                        b * MAT_ELEMS + 2 + 16 * ROW_STRIDE,
                        [[16 * ROW_STRIDE, NQ], [ROW_STRIDE, 15], [1, 32]],
                    ),
                )
                # prefix sums along the flat band: P[f] = sum_{f' <= f} BT[f']
                # (two independent chains; window differences never straddle)
                pp = p_pp.tile([NQ, 1024], f32, tag="pp")
                nc.gpsimd.memset(pp[:, 0:1], 0.0)
                nc.vector.tensor_tensor_scan(
                    out=pp[:, 1:512],
                    data0=bt[:, 1:512],
                    data1=bt[:, 1:512],
                    initial=0.0,
                    op0=add,
                    op1=bypass,
                )
                nc.vector.tensor_tensor_scan(
                    out=pp[:, 512:993],
                    data0=bt[:, 512:993],
                    data1=bt[:, 512:993],
                    initial=0.0,
                    op0=add,
                    op1=bypass,
                )
                # 16-wide window sums: S16[q, tau, m] = P[32tau+m+16] - P[32tau+m]
                s16 = p_s16.tile([NQ, 31 * 16], f32, tag="s16")
                nc.vector.tensor_tensor(
                    out=v(s16, 0, [[496, NQ], [16, 31], [1, 16]]),
                    in0=v(pp, 16, [[1024, NQ], [32, 31], [1, 16]]),
                    in1=v(pp, 0, [[1024, NQ], [32, 31], [1, 16]]),
                    op=sub,
                )
                # out[16q+u] = sum_s S16[q, u+s, 15-s]  (flat: 16u + 15s + 15)
                r = p_r.tile([NQ, 16], f32, tag="r")
                nc.vector.reduce_sum(
                    out=r[:, :],
                    in_=v(s16, 15, [[496, NQ], [16, 16], [15, 16]]),
                    axis=X,
                )
                ro = p_r.tile([NQ, 16], f32, tag="ro")
                nc.scalar.mul(ro[:, :], r[:, :], 1.0 / 256.0)
                # y[b, 16q+u] = ro[q, u]
                nc.sync.dma_start(
                    out=bass.AP(y, b * ND, [[16, NQ], [1, 16]]),
                    in_=v(ro, 0, [[16, NQ], [1, 16]]),
                )

    nc.compile()
    return nc


def _get_compiled():
    global _COMPILED
    if _COMPILED is None:
        _COMPILED = _build()
    return _COMPILED


def kernel(x: np.ndarray) -> np.ndarray:
    global LAST_EXEC_TIME_NS
    from concourse.bass_utils import run_bass_kernel_spmd

    x = np.ascontiguousarray(np.asarray(x), dtype=np.float32)
    assert x.shape == (B_FULL, MAT, MAT), x.shape

    nc = _get_compiled()
    in_maps = [
        {"x": x[i * B_PER_CORE : (i + 1) * B_PER_CORE]} for i in range(N_CORES)
    ]
    trace = bool(int(os.environ.get("KERNEL_TRACE", "0")))
    if trace:
        # test-only: keep NTFF artifacts local instead of uploading
        from concourse import bass_utils as _bu

        _bu.upload_artifacts = lambda tmpdir: tmpdir
    res = run_bass_kernel_spmd(
        nc, in_maps, core_ids=list(range(N_CORES)), trace=trace
    )
    LAST_EXEC_TIME_NS = res.exec_time_ns
    out = np.concatenate([res.results[i]["y"] for i in range(N_CORES)], axis=0)
    return out.astype(np.float32)


# revision 5
# speedup vs baseline: 1.1792x; 1.0779x over previous
"""Trainium2 Bass kernel for DiamondLayer.

Computes out[b, d] = mean(x[b, d:d+16, d+17:d+33]) for d in [0, 2016),
i.e. 16x16 mean-pool windows sliding along a diagonal.

Strategy:
  - Pure data parallel over batch: 32 batches -> 8 cores x 4 batches.
  - Only a narrow diagonal band of each 2048x2048 matrix is touched
    (cols [r+2, r+33] of row r).  Each core DMAs just that band
    (~500KB/batch instead of 16MB/batch) straight out of the full
    input in HBM using a strided access pattern.
  - Band layout: BT[q, tau, j] = x[b, 16q+tau, (16q+tau)+2+j],
    q in [0,126), tau in [0,31), j in [0,32).  Window d = 16q+u needs
    rows 16q+u .. 16q+u+15, all resident in partition q, so the whole
    reduction is per-partition with uniform access patterns:
      out[16q+u] = sum_{i,k} BT[q, u+i, 15-i+k]
    computed as a 2-level pairwise tree (w2, w4) along j followed by
    4 strided window reduces and a final scaled add.
"""

import os
import sys
import numpy as np

for _p in ("/opt/trn_rl_repo",):
    if _p not in sys.path:
        sys.path.insert(0, _p)

B_FULL = 32
N_CORES = 8
B_PER_CORE = B_FULL // N_CORES  # 4
MAT = 2048
DS = 16
ND = MAT - 2 * DS  # 2016
NQ = ND // 16  # 126 partition groups of 16 diamonds
ROW_STRIDE = MAT + 1  # 2049: distance between band starts of adjacent rows
MAT_ELEMS = MAT * MAT

LAST_EXEC_TIME_NS = None

_COMPILED = None


def _build():
    import concourse.bass as bass
    import concourse.bacc as bacc
    import concourse.tile as tile
    from concourse import mybir

    f32 = mybir.dt.float32
    add = mybir.AluOpType.add
    X = mybir.AxisListType.X

    nc = bacc.Bacc("TRN2", target_bir_lowering=False, debug=False)
    x = nc.dram_tensor("x", [B_PER_CORE, MAT, MAT], f32, kind="ExternalInput")
    y = nc.dram_tensor("y", [B_PER_CORE, ND], f32, kind="ExternalOutput")

    def v(t, off, pat):
        return bass.AP(t.tensor, off, pat)

    sub = mybir.AluOpType.subtract
    bypass = mybir.AluOpType.bypass

    with tile.TileContext(nc) as tc:
        with (
            tc.tile_pool(name="bt", bufs=2) as p_bt,
            tc.tile_pool(name="pp", bufs=2) as p_pp,
            tc.tile_pool(name="s16", bufs=2) as p_s16,
            tc.tile_pool(name="r", bufs=2) as p_r,
        ):
            for b in range(B_PER_CORE):
                # band load: BT[q, 1 + 32*tau + j] = x[b, 16q+tau, 16q+tau+2+j]
                # split at tau=16 so the scan of the first half can start
                # while the second half is still in flight.
                bt = p_bt.tile([NQ, 1024], f32, tag="bt")
                nc.sync.dma_start(
                    out=v(bt, 1, [[1024, NQ], [32, 16], [1, 32]]),
                    in_=bass.AP(
                        x,
                        b * MAT_ELEMS + 2,
                        [[16 * ROW_STRIDE, NQ], [ROW_STRIDE, 16], [1, 32]],
                    ),
                )
                nc.sync.dma_start(
                    out=v(bt, 1 + 512, [[1024, NQ], [32, 15], [1, 32]]),
                    in_=bass.AP(
                        x,
                        b * MAT_ELEMS + 2 + 16 * ROW_STRIDE,
                        [[16 * ROW_STRIDE, NQ], [ROW_STRIDE, 15], [1, 32]],
                    ),
                )
                # prefix sums along the flat band: P[f] = sum_{f' <= f} BT[f']
                # (two independent chains; window differences never straddle)
                pp = p_pp.tile([NQ, 1024], f32, tag="pp")
                nc.gpsimd.memset(pp[:, 0:1], 0.0)
                nc.vector.tensor_tensor_scan(
                    out=pp[:, 1:512],
                    data0=bt[:, 1:512],
                    data1=bt[:, 1:512],
                    initial=0.0,
                    op0=add,
                    op1=bypass,
                )
                nc.vector.tensor_tensor_scan(
                    out=pp[:, 512:993],
                    data0=bt[:, 512:993],
                    data1=bt[:, 512:993],
                    initial=0.0,
                    op0=add,
                    op1=bypass,
                )
                # 16-wide window sums: S16[q, tau, m] = P[32tau+m+16] - P[32tau+m]
                s16 = p_s16.tile([NQ, 31 * 16], f32, tag="s16")
                nc.vector.tensor_tensor(
                    out=v(s16, 0, [[496, NQ], [16, 31], [1, 16]]),
                    in0=v(pp, 16, [[1024, NQ], [32, 31], [1, 16]]),
                    in1=v(pp, 0, [[1024, NQ], [32, 31], [1, 16]]),
                    op=sub,
                )
                # out[16q+u] = sum_s S16[q, u+s, 15-s]  (flat: 16u + 15s + 15)
                r = p_r.tile([NQ, 16], f32, tag="r")
                nc.vector.reduce_sum(
                    out=r[:, :],
                    in_=v(s16, 15, [[496, NQ], [16, 16], [15, 16]]),
                    axis=X,
                )
                ro = p_r.tile([NQ, 16], f32, tag="ro")
                nc.scalar.mul(ro[:, :], r[:, :], 1.0 / 256.0)
                # y[b, 16q+u] = ro[q, u]
                nc.sync.dma_start(
                    out=bass.AP(y, b * ND, [[16, NQ], [1, 16]]),
                    in_=v(ro, 0, [[16, NQ], [1, 16]]),
                )

    nc.compile()
    return nc


def _get_compiled():
    global _COMPILED
    if _COMPILED is None:
        impl = os.environ.get("KERNEL_IMPL")
        if impl == "raw":  # dev-only toggle
            import kernel_raw

            _COMPILED = kernel_raw.build()
        elif impl == "raw2":
            import kernel_raw2

            _COMPILED = kernel_raw2.build()
        else:
            _COMPILED = _build()
    return _COMPILED


def kernel(x: np.ndarray) -> np.ndarray:
    global LAST_EXEC_TIME_NS
    from concourse.bass_utils import run_bass_kernel_spmd

    x = np.ascontiguousarray(np.asarray(x), dtype=np.float32)
    assert x.shape == (B_FULL, MAT, MAT), x.shape

    nc = _get_compiled()
    in_maps = [
        {"x": x[i * B_PER_CORE : (i + 1) * B_PER_CORE]} for i in range(N_CORES)
    ]
    trace = bool(int(os.environ.get("KERNEL_TRACE", "0")))
    if trace:
        # test-only: keep NTFF artifacts local instead of uploading
        from concourse import bass_utils as _bu

        _bu.upload_artifacts = lambda tmpdir: tmpdir
    res = run_bass_kernel_spmd(
        nc, in_maps, core_ids=list(range(N_CORES)), trace=trace
    )
    LAST_EXEC_TIME_NS = res.exec_time_ns
    out = np.concatenate([res.results[i]["y"] for i in range(N_CORES)], axis=0)
    return out.astype(np.float32)


# revision 6
# speedup vs baseline: 1.1983x; 1.0162x over previous
"""Trainium2 Bass kernel for DiamondLayer.

Computes out[b, d] = mean(x[b, d:d+16, d+17:d+33]) for d in [0, 2016),
i.e. 16x16 mean-pool windows sliding along a diagonal.

Strategy:
  - Pure data parallel over batch: 32 batches -> 8 cores x 4 batches.
  - Only a narrow diagonal band of each 2048x2048 matrix is touched
    (cols [r+2, r+33] of row r).  Each core DMAs just that band
    (~500KB/batch instead of 16MB/batch) straight out of the full
    input in HBM using a strided access pattern.
  - Band layout: BT[q, tau, j] = x[b, 16q+tau, (16q+tau)+2+j],
    q in [0,126), tau in [0,31), j in [0,32).  Window d = 16q+u needs
    rows 16q+u .. 16q+u+15, all resident in partition q, so the whole
    reduction is per-partition with uniform access patterns:
      out[16q+u] = sum_{i,k} BT[q, u+i, 15-i+k]
    computed as a 2-level pairwise tree (w2, w4) along j followed by
    4 strided window reduces and a final scaled add.
"""

import os
import sys
import numpy as np

for _p in ("/opt/trn_rl_repo",):
    if _p not in sys.path:
        sys.path.insert(0, _p)

B_FULL = 32
N_CORES = 8
B_PER_CORE = B_FULL // N_CORES  # 4
MAT = 2048
DS = 16
ND = MAT - 2 * DS  # 2016
NQ = ND // 16  # 126 partition groups of 16 diamonds
ROW_STRIDE = MAT + 1  # 2049: distance between band starts of adjacent rows
MAT_ELEMS = MAT * MAT

LAST_EXEC_TIME_NS = None

_COMPILED = None


def _build():
    import concourse.bass as bass
    import concourse.bacc as bacc
    import concourse.tile as tile
    from concourse import mybir

    f32 = mybir.dt.float32
    add = mybir.AluOpType.add
    X = mybir.AxisListType.X

    nc = bacc.Bacc("TRN2", target_bir_lowering=False, debug=False)
    x = nc.dram_tensor("x", [B_PER_CORE, MAT, MAT], f32, kind="ExternalInput")
    y = nc.dram_tensor("y", [B_PER_CORE, ND], f32, kind="ExternalOutput")

    def v(t, off, pat):
        return bass.AP(t.tensor, off, pat)

    sub = mybir.AluOpType.subtract
    bypass = mybir.AluOpType.bypass

    with tile.TileContext(nc) as tc:
        with (
            tc.tile_pool(name="bt", bufs=2) as p_bt,
            tc.tile_pool(name="pp", bufs=2) as p_pp,
            tc.tile_pool(name="s16", bufs=2) as p_s16,
            tc.tile_pool(name="r", bufs=2) as p_r,
        ):
            for b in range(B_PER_CORE):
                # band load: BT[q, 1 + 32*tau + j] = x[b, 16q+tau, 16q+tau+2+j]
                # split at tau=16 so the scan of the first half can start
                # while the second half is still in flight.
                bt = p_bt.tile([NQ, 1024], f32, tag="bt")
                nc.sync.dma_start(
                    out=v(bt, 1, [[1024, NQ], [32, 16], [1, 32]]),
                    in_=bass.AP(
                        x,
                        b * MAT_ELEMS + 2,
                        [[16 * ROW_STRIDE, NQ], [ROW_STRIDE, 16], [1, 32]],
                    ),
                )
                nc.sync.dma_start(
                    out=v(bt, 1 + 512, [[1024, NQ], [32, 15], [1, 32]]),
                    in_=bass.AP(
                        x,
                        b * MAT_ELEMS + 2 + 16 * ROW_STRIDE,
                        [[16 * ROW_STRIDE, NQ], [ROW_STRIDE, 15], [1, 32]],
                    ),
                )
                # prefix sums along the flat band: P[f] = sum_{f' <= f} BT[f']
                # (two independent chains; window differences never straddle)
                pp = p_pp.tile([NQ, 1024], f32, tag="pp")
                nc.gpsimd.memset(pp[:, 0:1], 0.0)
                nc.vector.tensor_tensor_scan(
                    out=pp[:, 1:512],
                    data0=bt[:, 1:512],
                    data1=bt[:, 1:512],
                    initial=0.0,
                    op0=add,
                    op1=bypass,
                )
                nc.vector.tensor_tensor_scan(
                    out=pp[:, 512:993],
                    data0=bt[:, 512:993],
                    data1=bt[:, 512:993],
                    initial=0.0,
                    op0=add,
                    op1=bypass,
                )
                # 16-wide window sums: S16[q, tau, m] = P[32tau+m+16] - P[32tau+m]
                s16 = p_s16.tile([NQ, 31 * 16], f32, tag="s16")
                nc.vector.tensor_tensor(
                    out=v(s16, 0, [[496, NQ], [16, 31], [1, 16]]),
                    in0=v(pp, 16, [[1024, NQ], [32, 31], [1, 16]]),
                    in1=v(pp, 0, [[1024, NQ], [32, 31], [1, 16]]),
                    op=sub,
                )
                # out[16q+u] = sum_s S16[q, u+s, 15-s]  (flat: 16u + 15s + 15)
                r = p_r.tile([NQ, 16], f32, tag="r")
                nc.vector.reduce_sum(
                    out=r[:, :],
                    in_=v(s16, 15, [[496, NQ], [16, 16], [15, 16]]),
                    axis=X,
                )
                ro = p_r.tile([NQ, 16], f32, tag="ro")
                nc.scalar.mul(ro[:, :], r[:, :], 1.0 / 256.0)
                # y[b, 16q+u] = ro[q, u]
                nc.sync.dma_start(
                    out=bass.AP(y, b * ND, [[16, NQ], [1, 16]]),
                    in_=v(ro, 0, [[16, NQ], [1, 16]]),
                )

    nc.compile()
    return nc


def _get_compiled():
    global _COMPILED
    if _COMPILED is None:
        impl = os.environ.get("KERNEL_IMPL")
        if impl == "raw":  # dev-only toggle
            import kernel_raw

            _COMPILED = kernel_raw.build()
        elif impl == "raw2":
            import kernel_raw2

            _COMPILED = kernel_raw2.build()
        elif impl == "raw3":
            import kernel_raw3

            _COMPILED = kernel_raw3.build()
        else:
            _COMPILED = _build()
    return _COMPILED


def kernel(x: np.ndarray) -> np.ndarray:
    global LAST_EXEC_TIME_NS
    from concourse.bass_utils import run_bass_kernel_spmd

    x = np.ascontiguousarray(np.asarray(x), dtype=np.float32)
    assert x.shape == (B_FULL, MAT, MAT), x.shape

    nc = _get_compiled()
    in_maps = [
        {"x": x[i * B_PER_CORE : (i + 1) * B_PER_CORE]} for i in range(N_CORES)
    ]
    trace = bool(int(os.environ.get("KERNEL_TRACE", "0")))
    if trace:
        # test-only: keep NTFF artifacts local instead of uploading
        from concourse import bass_utils as _bu

        _bu.upload_artifacts = lambda tmpdir: tmpdir
    res = run_bass_kernel_spmd(
        nc, in_maps, core_ids=list(range(N_CORES)), trace=trace
    )
    LAST_EXEC_TIME_NS = res.exec_time_ns
    out = np.concatenate([res.results[i]["y"] for i in range(N_CORES)], axis=0)
    return out.astype(np.float32)
